# revision 13
# baseline (speedup 1.0000x reference)
"""Trainium2 Bass kernel for nn_AttnGlobal (B=8, N=4096, DIM=128).

reference:
    kv = x @ Wkv + bkv ; k, v = split(kv)
    q = q_global / sqrt(d)
    scores = einsum("bnd,bmd->bnm", k, q)       # softmax over m
    attn = softmax(scores, axis=-1)
    out = einsum("bnm,bmd->bnd", attn, v) @ Wp + bp

Sharding: pure data-parallel over B across the 8 cores (one batch each).

Host-side algebra folds:
    w   = x @ (Wv @ Wp)            (since attn @ (x@Wv) @ Wp = attn @ (x@(Wv@Wp)))
    bpe = bv @ Wp + bp             (rows of attn sum to 1; added on host)

Per-core dataflow (outputs kept transposed; host divides + transposes back):
    xT, qT  : host-pretransposed fp16                [d, n] / [d, m]
    kT      = Wk.T @ xT + bk                         [d, n]   fp16
    S.T     = qT.T-chunks @ kT                       [m, n] tiles in PSUM (fp32)
    E.T     = exp(S.T / sqrt(d))                     fp16, ACT straight from PSUM
    U.T     = sum_t w_t.T @ E.T_t   (per n-chunk)    [d, n] PSUM accum, 32 wide mms
    P       = pair-tree sum of E.T tiles over t      [m-part, n] fp16 (DVE)
    D       = ones.T @ P                             [1, n] PSUM (one mm per chunk)
    host    : out = (U.T / D).T + bpe

The U.T formulation (w stationary, E.T moving, 512-wide) replaces the
previous 129-wide augmented scheme: 32+1 matmuls per chunk instead of 128,
relieving the PE sequencer (~112 ns/matmul dispatch) which was the
critical resource alongside the ACT engine.
"""

import os
import sys

try:
    import concourse  # noqa: F401  (resolvable via PYTHONPATH on axon images)
except ImportError:
    for _p in ("/opt/trn_rl_repo", os.path.expanduser("~/.axon_site/_ro/trn_rl_repo")):
        if os.path.isdir(_p) and _p not in sys.path:
            sys.path.append(_p)

from collections import deque

import numpy as np

import concourse.bacc as bacc
import concourse.mybir as mybir
from concourse.bass_utils import run_bass_kernel_spmd
from concourse.tile import TileContext

B, N, D = 8, 4096, 128
NT = N // 128          # 32 row tiles
NC = N // 512          # 8 column chunks
F32 = mybir.dt.float32
F16 = mybir.dt.float16
EXP_SCALE = 1.0 / float(np.sqrt(D))

# alternating PSUM score-group sizes; sum == NT, st4 uses 4 banks, st2 uses 2
S_GROUPS = [2, 4, 2, 4, 2, 4, 2, 4, 2, 4, 2]
assert sum(S_GROUPS) == NT


def build(reps: int = 1):
    """Build and compile the per-core Bass program (identical on all cores)."""
    nc = bacc.Bacc("TRN2", target_bir_lowering=False)

    xt = nc.dram_tensor("xt", [D, N], F16, kind="ExternalInput")
    qt = nc.dram_tensor("qt", [D, N], F16, kind="ExternalInput")
    wk = nc.dram_tensor("wk", [D, D], F16, kind="ExternalInput")
    wvp = nc.dram_tensor("wvp", [D, D], F16, kind="ExternalInput")
    bk = nc.dram_tensor("bk", [D, 1], F32, kind="ExternalInput")
    ot = nc.dram_tensor("ot", [D, N], F16, kind="ExternalOutput")
    od = nc.dram_tensor("od", [1, N], F32, kind="ExternalOutput")

    with TileContext(nc) as tc:
        xt_sb = nc.alloc_sbuf_tensor("xt_sb", [128, N], F16)
        qt_sb = nc.alloc_sbuf_tensor("qt_sb", [128, N], F16)
        kt_sb = nc.alloc_sbuf_tensor("kt_sb", [128, N], F16)
        w_sb = nc.alloc_sbuf_tensor("w_sb", [128, NT, 128], F16)
        ET = [nc.alloc_sbuf_tensor(f"et{i}", [128, NT, 512], F16) for i in range(3)]
        PT = [nc.alloc_sbuf_tensor(f"pt{i}", [128, 16, 512], F16) for i in range(2)]
        wk_sb = nc.alloc_sbuf_tensor("wk_sb", [128, 128], F16)
        wvp_sb = nc.alloc_sbuf_tensor("wvp_sb", [128, 128], F16)
        bk_sb = nc.alloc_sbuf_tensor("bk_sb", [128, 1], F32)
        ones_sb = nc.alloc_sbuf_tensor("ones_sb", [128, 128], F16)

        with (
            tc.tile_pool(name="outp", bufs=3) as outp,
            tc.tile_pool(name="ps", bufs=1, space="PSUM") as psp,
            tc.tile_pool(name="st4", bufs=1, space="PSUM") as st4,
            tc.tile_pool(name="st2", bufs=1, space="PSUM") as st2,
        ):

            def s_group(c, mt, g, pool=None, tag=None):
                """scores S.T [m-tiles mt..mt+g, n-chunk c] -> exp -> E.T
                plus DVE pair-sums + streamed reduction-tree cascade into
                PT[c % 2] for the denominator."""
                if pool is None:
                    pool = st4 if g == 4 else st2
                    tag = f"st{g}"
                stp = pool.tile([128, g * 512], F32, tag=tag)
                for i in range(g):
                    m = mt + i
                    nc.tensor.matmul(
                        stp[:, i * 512:(i + 1) * 512],
                        qt_sb[:, m * 128:(m + 1) * 128],
                        kt_sb[:, c * 512:(c + 1) * 512],
                    )
                buf = ET[c % 3]
                nc.scalar.activation(
                    buf[:, mt:mt + g, :],
                    stp[:],
                    mybir.ActivationFunctionType.Exp,
                    scale=EXP_SCALE,
                )
                # streamed denominator tree, multi-tile adds: each completed
                # 8-tile block collapses in 3 instrs (4+2+1 tiles wide), then
                # block sums combine at the 16/32-tile crossings. 15 DVE
                # instrs per chunk instead of 31, same element volume.
                pt = PT[c % 2]
                lo, hi = mt, mt + g
                for k in range(lo // 8, hi // 8):
                    s = 4 * k
                    nc.vector.tensor_add(
                        pt[:, s:s + 4, :], buf[:, 8 * k:8 * k + 4, :],
                        buf[:, 8 * k + 4:8 * k + 8, :],
                    )
                    nc.vector.tensor_add(
                        pt[:, s:s + 2, :], pt[:, s:s + 2, :], pt[:, s + 2:s + 4, :]
                    )
                    nc.vector.tensor_add(
                        pt[:, s, :], pt[:, s, :], pt[:, s + 1, :]
                    )
                if lo < 16 <= hi:
                    nc.vector.tensor_add(pt[:, 0, :], pt[:, 0, :], pt[:, 4, :])
                if hi == NT:
                    nc.vector.tensor_add(pt[:, 8, :], pt[:, 8, :], pt[:, 12, :])
                    nc.vector.tensor_add(pt[:, 0, :], pt[:, 0, :], pt[:, 8, :])

            GROUPS = []
            _mt = 0
            for _g in S_GROUPS:
                GROUPS.append((_mt, _g))
                _mt += _g

            def u_group(c, mt, g, psu):
                # U.T[d, n-chunk] += w_t.T @ E.T_t for the group's tiles
                buf = ET[c % 3]
                for i in range(g):
                    t = mt + i
                    nc.tensor.matmul(
                        psu[:],
                        w_sb[:, t, :],
                        buf[:, t, :],
                        start=(t == 0),
                        stop=(t == NT - 1),
                    )

            def epilogue(c, psu):
                # cross-partition sum of the streamed tree result, then ship
                # U.T (via fp16 cast) and D (straight from PSUM) out.
                psd = psp.tile([128, 512], F32, tag="m")
                nc.tensor.matmul(psd[0:1, :], ones_sb[:, 0:1], PT[c % 2][:, 0, :])
                dsb = outp.tile([128, 512], F32, tag="d")
                nc.vector.tensor_copy(dsb[0:1, :], psd[0:1, :])
                nc.sync.dma_start(od[0:1, c * 512:(c + 1) * 512], dsb[0:1, :])
                o = outp.tile([128, 512], F16, tag="o")
                nc.vector.tensor_copy(o[:], psu[:])
                nc.sync.dma_start(ot[:, c * 512:(c + 1) * 512], o[:])

            nc.vector.memset(ones_sb[:], 1.0)

            def body(_iv=None):
                # phase 1: stream inputs (halves), compute kT and w per chunk;
                # S(0) groups dribble in as their dependencies land.
                nc.sync.dma_start(wk_sb[:], wk[:])
                nc.sync.dma_start(bk_sb[:], bk[:])
                nc.sync.dma_start(xt_sb[:, :512], xt[:, :512])
                nc.sync.dma_start(qt_sb[:, :512], qt[:, :512])
                nc.sync.dma_start(xt_sb[:, 512: N // 2], xt[:, 512: N // 2])
                nc.sync.dma_start(qt_sb[:, 512: N // 2], qt[:, 512: N // 2])
                nc.sync.dma_start(wvp_sb[:], wvp[:])
                nc.sync.dma_start(xt_sb[:, N // 2:], xt[:, N // 2:])
                nc.sync.dma_start(qt_sb[:, N // 2:], qt[:, N // 2:])
                sg = 0
                mt_done = 0
                for c in range(NC):
                    kt = psp.tile([128, 512], F32, tag="m")
                    nc.tensor.matmul(kt[:], wk_sb[:], xt_sb[:, c * 512:(c + 1) * 512])
                    nc.vector.tensor_scalar_add(
                        kt_sb[:, c * 512:(c + 1) * 512], kt[:], bk_sb[:]
                    )
                    while sg < len(S_GROUPS) and mt_done + S_GROUPS[sg] <= (c + 1) * 4:
                        s_group(0, mt_done, S_GROUPS[sg])
                        mt_done += S_GROUPS[sg]
                        sg += 1
                    wp = psp.tile([128, 512], F32, tag="u")
                    for i in range(4):
                        nc.tensor.matmul(
                            wp[:, i * 128:(i + 1) * 128],
                            xt_sb[:, (c * 4 + i) * 128:(c * 4 + i + 1) * 128],
                            wvp_sb[:],
                        )
                    nc.vector.tensor_copy(w_sb[:, 4 * c:4 * c + 4, :], wp[:])
                # phase 2: serial s_phase(c+1) / u_phase(c) pipeline; the
                # denominator tree is already streamed inside s_group, so
                # u_phase is just the 32 U matmuls + epilogue.
                for c in range(NC):
                    if c + 1 < NC:
                        if c + 1 == NC - 1:
                            # drain mode: no S(c+2) will hide slot turnaround,
                            # use 16 groups of 2 ping-ponging across both pools
                            for i in range(16):
                                pool, tag = (
                                    (st4, "st4") if i % 2 == 0 else (st2, "st2")
                                )
                                s_group(c + 1, i * 2, 2, pool=pool, tag=tag)
                        else:
                            for mt, g in GROUPS:
                                s_group(c + 1, mt, g)
                    psu = psp.tile([128, 512], F32, tag="u", name=f"psu{c}")
                    u_group(c, 0, NT, psu)
                    epilogue(c, psu)

            if reps == 1:
                body()
            else:
                with tc.For_i(0, reps, 1):
                    body()

    nc.compile()
    return nc


def _prep_weights(Wkv, bkv, Wp, bp):
    Wkv = np.asarray(Wkv, np.float32)
    bkv = np.asarray(bkv, np.float32)
    Wp = np.asarray(Wp, np.float32)
    bp = np.asarray(bp, np.float32)
    wk = np.ascontiguousarray(Wkv[:, :D].astype(np.float16))
    bk = np.ascontiguousarray(bkv[:D]).reshape(D, 1)
    wvp = np.ascontiguousarray((Wkv[:, D:] @ Wp).astype(np.float16))
    bpe_row = bkv[D:] @ Wp + bp
    return wk, bk, wvp, bpe_row


_NC_CACHE = {}


def kernel(x, q_global, Wkv, bkv, Wp, bp):
    xt = np.asarray(x, np.float32).astype(np.float16).transpose(0, 2, 1)
    qt = np.asarray(q_global, np.float32).astype(np.float16).transpose(0, 2, 1)
    wk, bk, wvp, bpe_row = _prep_weights(Wkv, bkv, Wp, bp)

    if 1 not in _NC_CACHE:
        _NC_CACHE[1] = build(reps=1)
    nc = _NC_CACHE[1]

    in_maps = [
        {
            "xt": np.ascontiguousarray(xt[b]),
            "qt": np.ascontiguousarray(qt[b]),
            "wk": wk,
            "wvp": wvp,
            "bk": bk,
        }
        for b in range(B)
    ]
    res = run_bass_kernel_spmd(nc, in_maps, core_ids=list(range(B)))
    outs = []
    for b in range(B):
        ut = res.results[b]["ot"].astype(np.float32)      # [d, n]
        dd = res.results[b]["od"].astype(np.float32)      # [1, n]
        outs.append((ut / dd).T + bpe_row)
    return np.stack(outs, axis=0)


# revision 15
# speedup vs baseline: 1.1487x; 1.1487x over previous
"""Trainium2 Bass kernel for nn_AttnGlobal (B=8, N=4096, DIM=128).

reference:
    kv = x @ Wkv + bkv ; k, v = split(kv)
    q = q_global / sqrt(d)
    scores = einsum("bnd,bmd->bnm", k, q)       # softmax over m
    attn = softmax(scores, axis=-1)
    out = einsum("bnm,bmd->bnd", attn, v) @ Wp + bp

Sharding: pure data-parallel over B across the 8 cores (one batch each).

Host-side algebra folds:
    w   = x @ (Wv @ Wp)            (since attn @ (x@Wv) @ Wp = attn @ (x@(Wv@Wp)))
    bpe = bv @ Wp + bp             (rows of attn sum to 1; added on host)

Per-core dataflow (outputs kept transposed; host divides + transposes back):
    xT, qT  : host-pretransposed fp16                [d, n] / [d, m]
    kT      = Wk.T @ xT + bk                         [d, n]   fp16
    S.T     = qT.T-chunks @ kT                       [m, n] tiles in PSUM (fp32)
    E.T     = exp(S.T / sqrt(d))                     fp16, ACT straight from PSUM
    U.T     = sum_t w_t.T @ E.T_t   (per n-chunk)    [d, n] PSUM accum, 32 wide mms
    P       = pair-tree sum of E.T tiles over t      [m-part, n] fp16 (DVE)
    D       = ones.T @ P                             [1, n] PSUM (one mm per chunk)
    host    : out = (U.T / D).T + bpe

The U.T formulation (w stationary, E.T moving, 512-wide) replaces the
previous 129-wide augmented scheme: 32+1 matmuls per chunk instead of 128,
relieving the PE sequencer (~112 ns/matmul dispatch) which was the
critical resource alongside the ACT engine.
"""

import os
import sys

try:
    import concourse  # noqa: F401  (resolvable via PYTHONPATH on axon images)
except ImportError:
    for _p in ("/opt/trn_rl_repo", os.path.expanduser("~/.axon_site/_ro/trn_rl_repo")):
        if os.path.isdir(_p) and _p not in sys.path:
            sys.path.append(_p)

from collections import deque

import numpy as np

import concourse.bacc as bacc
import concourse.mybir as mybir
from concourse.bass_utils import run_bass_kernel_spmd
from concourse.tile import TileContext

B, N, D = 8, 4096, 128
NT = N // 128          # 32 row tiles
NC = N // 512          # 8 column chunks
F32 = mybir.dt.float32
F16 = mybir.dt.float16
EXP_SCALE = 1.0 / float(np.sqrt(D))

# alternating PSUM score-group sizes; sum == NT, st4 uses 4 banks, st2 uses 2
S_GROUPS = [2, 4, 2, 4, 2, 4, 2, 4, 2, 4, 2]
assert sum(S_GROUPS) == NT


def build(reps: int = 1):
    """Build and compile the per-core Bass program (identical on all cores)."""
    nc = bacc.Bacc("TRN2", target_bir_lowering=False)

    xt = nc.dram_tensor("xt", [D, N], F16, kind="ExternalInput")
    qt = nc.dram_tensor("qt", [D, N], F16, kind="ExternalInput")
    wk = nc.dram_tensor("wk", [D, D], F16, kind="ExternalInput")
    wvp = nc.dram_tensor("wvp", [D, D], F16, kind="ExternalInput")
    bk = nc.dram_tensor("bk", [D, 1], F32, kind="ExternalInput")
    ot = nc.dram_tensor("ot", [D, N], F16, kind="ExternalOutput")
    od = nc.dram_tensor("od", [1, N], F32, kind="ExternalOutput")

    with TileContext(nc) as tc:
        xt_sb = nc.alloc_sbuf_tensor("xt_sb", [128, N], F16)
        qt_sb = nc.alloc_sbuf_tensor("qt_sb", [128, N], F16)
        kt_sb = nc.alloc_sbuf_tensor("kt_sb", [128, N], F16)
        w_sb = nc.alloc_sbuf_tensor("w_sb", [128, NT, 128], F16)
        ET = [nc.alloc_sbuf_tensor(f"et{i}", [128, NT, 512], F16) for i in range(3)]
        PT = [nc.alloc_sbuf_tensor(f"pt{i}", [128, 16, 512], F16) for i in range(2)]
        wk_sb = nc.alloc_sbuf_tensor("wk_sb", [128, 128], F16)
        wvp_sb = nc.alloc_sbuf_tensor("wvp_sb", [128, 128], F16)
        bk_sb = nc.alloc_sbuf_tensor("bk_sb", [128, 1], F32)
        ones_sb = nc.alloc_sbuf_tensor("ones_sb", [128, 128], F16)

        with (
            tc.tile_pool(name="outp", bufs=3) as outp,
            tc.tile_pool(name="ps", bufs=1, space="PSUM") as psp,
            tc.tile_pool(name="st4", bufs=1, space="PSUM") as st4,
            tc.tile_pool(name="st2", bufs=1, space="PSUM") as st2,
        ):

            def s_group(c, mt, g, pool=None, tag=None):
                """scores S.T [m-tiles mt..mt+g, n-chunk c] -> exp -> E.T
                plus DVE pair-sums + streamed reduction-tree cascade into
                PT[c % 2] for the denominator."""
                if pool is None:
                    pool = st4 if g == 4 else st2
                    tag = f"st{g}"
                stp = pool.tile([128, g * 512], F32, tag=tag)
                for i in range(g):
                    m = mt + i
                    nc.tensor.matmul(
                        stp[:, i * 512:(i + 1) * 512],
                        qt_sb[:, m * 128:(m + 1) * 128],
                        kt_sb[:, c * 512:(c + 1) * 512],
                    )
                buf = ET[c % 3]
                nc.scalar.activation(
                    buf[:, mt:mt + g, :],
                    stp[:],
                    mybir.ActivationFunctionType.Exp,
                    scale=EXP_SCALE,
                )
                # streamed denominator tree, multi-tile adds: each completed
                # 8-tile block collapses in 3 instrs (4+2+1 tiles wide), then
                # block sums combine at the 16/32-tile crossings. 15 DVE
                # instrs per chunk instead of 31, same element volume.
                pt = PT[c % 2]
                lo, hi = mt, mt + g
                for k in range(lo // 8, hi // 8):
                    s = 4 * k
                    nc.vector.tensor_add(
                        pt[:, s:s + 4, :], buf[:, 8 * k:8 * k + 4, :],
                        buf[:, 8 * k + 4:8 * k + 8, :],
                    )
                    nc.vector.tensor_add(
                        pt[:, s:s + 2, :], pt[:, s:s + 2, :], pt[:, s + 2:s + 4, :]
                    )
                    nc.vector.tensor_add(
                        pt[:, s, :], pt[:, s, :], pt[:, s + 1, :]
                    )
                if lo < 16 <= hi:
                    nc.vector.tensor_add(pt[:, 0, :], pt[:, 0, :], pt[:, 4, :])
                if hi == NT:
                    nc.vector.tensor_add(pt[:, 8, :], pt[:, 8, :], pt[:, 12, :])
                    nc.vector.tensor_add(pt[:, 0, :], pt[:, 0, :], pt[:, 8, :])

            GROUPS = []
            _mt = 0
            for _g in S_GROUPS:
                GROUPS.append((_mt, _g))
                _mt += _g

            def u_group(c, mt, g, psu):
                # U.T[d, n-chunk] += w_t.T @ E.T_t for the group's tiles
                buf = ET[c % 3]
                for i in range(g):
                    t = mt + i
                    nc.tensor.matmul(
                        psu[:],
                        w_sb[:, t, :],
                        buf[:, t, :],
                        start=(t == 0),
                        stop=(t == NT - 1),
                    )

            def epilogue(c, psu):
                # cross-partition sum of the streamed tree result, then ship
                # U.T (via fp16 cast) and D (straight from PSUM) out.
                psd = psp.tile([128, 512], F32, tag="m")
                nc.tensor.matmul(psd[0:1, :], ones_sb[:, 0:1], PT[c % 2][:, 0, :])
                dsb = outp.tile([128, 512], F32, tag="d")
                nc.vector.tensor_copy(dsb[0:1, :], psd[0:1, :])
                nc.scalar.dma_start(od[0:1, c * 512:(c + 1) * 512], dsb[0:1, :])
                o = outp.tile([128, 512], F16, tag="o")
                nc.vector.tensor_copy(o[:], psu[:])
                nc.sync.dma_start(ot[:, c * 512:(c + 1) * 512], o[:])

            nc.vector.memset(ones_sb[:], 1.0)

            def body(_iv=None):
                # phase 1: stream inputs (halves), compute kT and w per chunk;
                # S(0) groups dribble in as their dependencies land.
                nc.sync.dma_start(wk_sb[:], wk[:])
                nc.sync.dma_start(bk_sb[:], bk[:])
                nc.sync.dma_start(xt_sb[:, :512], xt[:, :512])
                nc.scalar.dma_start(qt_sb[:, :512], qt[:, :512])
                nc.sync.dma_start(xt_sb[:, 512: N // 2], xt[:, 512: N // 2])
                nc.scalar.dma_start(qt_sb[:, 512: N // 2], qt[:, 512: N // 2])
                nc.sync.dma_start(wvp_sb[:], wvp[:])
                nc.sync.dma_start(xt_sb[:, N // 2:], xt[:, N // 2:])
                nc.scalar.dma_start(qt_sb[:, N // 2:], qt[:, N // 2:])
                sg = 0
                mt_done = 0
                for c in range(NC):
                    kt = psp.tile([128, 512], F32, tag="m")
                    nc.tensor.matmul(kt[:], wk_sb[:], xt_sb[:, c * 512:(c + 1) * 512])
                    nc.vector.tensor_scalar_add(
                        kt_sb[:, c * 512:(c + 1) * 512], kt[:], bk_sb[:]
                    )
                    while sg < len(S_GROUPS) and mt_done + S_GROUPS[sg] <= (c + 1) * 4:
                        s_group(0, mt_done, S_GROUPS[sg])
                        mt_done += S_GROUPS[sg]
                        sg += 1
                    wp = psp.tile([128, 512], F32, tag="u")
                    for i in range(4):
                        nc.tensor.matmul(
                            wp[:, i * 128:(i + 1) * 128],
                            xt_sb[:, (c * 4 + i) * 128:(c * 4 + i + 1) * 128],
                            wvp_sb[:],
                        )
                    nc.vector.tensor_copy(w_sb[:, 4 * c:4 * c + 4, :], wp[:])
                # phase 2: serial s_phase(c+1) / u_phase(c) pipeline; the
                # denominator tree is already streamed inside s_group, so
                # u_phase is just the 32 U matmuls + epilogue.
                for c in range(NC):
                    if c + 1 < NC:
                        if c + 1 == NC - 1:
                            # drain mode: no S(c+2) will hide slot turnaround,
                            # use 16 groups of 2 ping-ponging across both pools
                            for i in range(16):
                                pool, tag = (
                                    (st4, "st4") if i % 2 == 0 else (st2, "st2")
                                )
                                s_group(c + 1, i * 2, 2, pool=pool, tag=tag)
                        else:
                            for mt, g in GROUPS:
                                s_group(c + 1, mt, g)
                    psu = psp.tile([128, 512], F32, tag="u", name=f"psu{c}")
                    u_group(c, 0, NT, psu)
                    epilogue(c, psu)

            if reps == 1:
                body()
            elif reps % 2 == 0:
                # two bodies per hardware-loop iteration: consecutive bodies
                # overlap as straight-line code even if the loop edge syncs
                with tc.For_i(0, reps // 2, 1):
                    body()
                    body()
            else:
                with tc.For_i(0, reps, 1):
                    body()

    nc.compile()
    return nc


def _prep_weights(Wkv, bkv, Wp, bp):
    Wkv = np.asarray(Wkv, np.float32)
    bkv = np.asarray(bkv, np.float32)
    Wp = np.asarray(Wp, np.float32)
    bp = np.asarray(bp, np.float32)
    wk = np.ascontiguousarray(Wkv[:, :D].astype(np.float16))
    bk = np.ascontiguousarray(bkv[:D]).reshape(D, 1)
    wvp = np.ascontiguousarray((Wkv[:, D:] @ Wp).astype(np.float16))
    bpe_row = bkv[D:] @ Wp + bp
    return wk, bk, wvp, bpe_row


_NC_CACHE = {}


def kernel(x, q_global, Wkv, bkv, Wp, bp):
    xt = np.asarray(x, np.float32).astype(np.float16).transpose(0, 2, 1)
    qt = np.asarray(q_global, np.float32).astype(np.float16).transpose(0, 2, 1)
    wk, bk, wvp, bpe_row = _prep_weights(Wkv, bkv, Wp, bp)

    if 1 not in _NC_CACHE:
        _NC_CACHE[1] = build(reps=1)
    nc = _NC_CACHE[1]

    in_maps = [
        {
            "xt": np.ascontiguousarray(xt[b]),
            "qt": np.ascontiguousarray(qt[b]),
            "wk": wk,
            "wvp": wvp,
            "bk": bk,
        }
        for b in range(B)
    ]
    res = run_bass_kernel_spmd(nc, in_maps, core_ids=list(range(B)))
    outs = []
    for b in range(B):
        ut = res.results[b]["ot"].astype(np.float32)      # [d, n]
        dd = res.results[b]["od"].astype(np.float32)      # [1, n]
        outs.append((ut / dd).T + bpe_row)
    return np.stack(outs, axis=0)
